# revision 3
# baseline (speedup 1.0000x reference)
"""EMA head kernel for Trainium2 (Bass/Tile), 8 NeuronCores.

Problem: alpha = clip(sigmoid(MLP(feat)), 0.01, 0.99) per (t, b);
         y[0] = r[0]; y[t] = (1-alpha[t])*y[t-1] + alpha[t]*r[t].

Sharding: time dim T=4096 split into 8 slabs of 512 (all B=256 per core).
Each core computes, for its slab, the local affine-scan pieces
    z[t] = A[t]*z[t-1] + Bv[t]   (z[-1] = 0),   A = 1-alpha, Bv = alpha*r
    P[t] = A[t]*P[t-1]           (P[-1] = 1)
and the host stitches slabs with   y = z + P * carry,  carry' = y[-1].
carry_0 = r[0] reproduces y[0] = r[0] exactly: a*r + (1-a)*r = r.

v2 layout: feat is cast to fp16 on the host (halves HBM traffic; the
MLP ran in fp16 already) and kept in natural [t*b, f] order in DRAM.
On-chip it is loaded with HWDGE xbar DMA-transpose straight into
featT [f=128, t*b] SBUF tiles — no PE transposes, no PSUM->SBUF copies.
Each 128-column (t,b)-chunk is one matmul lhsT against W1 (rhs), giving
h [128 bt, 16] in PSUM, collected 32 t-steps per PSUM bank, then
+b1/relu/*W2/reduce on DVE+ACT -> alpha_pre [128 b, t], sigmoid+clip,
and tensor_tensor_scan along the free (t) dim for z and P.
r is transposed host-side to rT [2, 128, t].
"""

import numpy as np

T, B, FEAT, HID = 4096, 256, 128, 16
NCORES = 8
TLOC = T // NCORES  # 512
NH = 2              # batch halves of 128
CH = 16384          # (t,b) columns per feat DMA-transpose chunk (64 t-steps, 4 MB)
NCHUNK = TLOC * B // CH  # 8

_CACHE = {}


def _build_program():
    import concourse.bacc as bacc
    import concourse.bass as bass
    import concourse.tile as tile
    from concourse import mybir

    fp32 = mybir.dt.float32
    fp16 = mybir.dt.float16
    AF = mybir.ActivationFunctionType
    OP = mybir.AluOpType

    nc = bacc.Bacc("TRN2", target_bir_lowering=False, debug=False,
                   num_devices=NCORES)

    feat_d = nc.dram_tensor("feat", [TLOC * B, FEAT], fp16, kind="ExternalInput")
    rt_d = nc.dram_tensor("rt", [NH, 128, TLOC], fp32, kind="ExternalInput")
    w1_d = nc.dram_tensor("w1", [FEAT, HID], fp16, kind="ExternalInput")
    b1_d = nc.dram_tensor("b1", [HID], fp32, kind="ExternalInput")
    w2_d = nc.dram_tensor("w2", [HID], fp32, kind="ExternalInput")
    b2_d = nc.dram_tensor("b2", [1], fp32, kind="ExternalInput")
    z_d = nc.dram_tensor("z", [NH, 128, TLOC], fp32, kind="ExternalOutput")
    p_d = nc.dram_tensor("p", [NH, 128, TLOC], fp32, kind="ExternalOutput")

    with tile.TileContext(nc) as tc:
        with (
            tc.tile_pool(name="singles", bufs=1) as singles,
            tc.tile_pool(name="featin", bufs=3) as featin,
            tc.tile_pool(name="hps", bufs=2, space="PSUM") as hps,
            tc.tile_pool(name="hwork", bufs=2) as hwork,
        ):
            # ---------------- constants / small inputs ----------------
            w1_sb = singles.tile([128, HID], fp16)
            nc.gpsimd.dma_start(w1_sb, w1_d[:, :])
            b1rep = singles.tile([128, 32, HID], fp32)
            nc.gpsimd.dma_start(
                b1rep, bass.AP(b1_d, 0, [[0, 128], [0, 32], [1, HID]]))
            w2rep = singles.tile([128, 32, HID], fp32)
            nc.gpsimd.dma_start(
                w2rep, bass.AP(w2_d, 0, [[0, 128], [0, 32], [1, HID]]))
            b2col = singles.tile([128, 1], fp32)
            nc.gpsimd.dma_start(b2col, bass.AP(b2_d, 0, [[0, 128], [1, 1]]))
            ones_sb = singles.tile([128, TLOC], fp32)
            nc.vector.memset(ones_sb, 1.0)

            rT = [singles.tile([128, TLOC], fp32, tag=f"rT{h}", name=f"rT{h}")
                  for h in range(NH)]
            for h in range(NH):
                nc.scalar.dma_start(rT[h], rt_d[h])

            # per-half alpha_pre accumulators [128 b, t]
            apre = [singles.tile([128, TLOC], fp32, tag=f"apre{h}", name=f"apre{h}")
                    for h in range(NH)]

            # ---------------- main feat pipeline ----------------
            TCH = CH // B  # t-steps per chunk (64)
            for k in range(NCHUNK):
                ft = featin.tile([128, CH], fp16, tag="ft")
                nc.sync.dma_start(ft, feat_d[k * CH:(k + 1) * CH, :],
                                  transpose=True)
                # 32-t blocks within this chunk
                for blk in range(TCH // 32):
                    hbank = [hps.tile([128, 32, HID], fp32, tag=f"h{h}",
                                      name=f"hbank{h}")
                             for h in range(NH)]
                    for tt in range(32):
                        col = (blk * 32 + tt) * B
                        for h in range(NH):
                            nc.tensor.matmul(
                                hbank[h][:, tt, :],
                                ft[:, col + h * 128:col + (h + 1) * 128],
                                w1_sb)
                    t0 = k * TCH + blk * 32
                    for h in range(NH):
                        hb = hwork.tile([128, 32, HID], fp32, tag="hb")
                        nc.vector.tensor_add(hb, hbank[h], b1rep)
                        hrelu = hwork.tile([128, 32, HID], fp32, tag="hrelu")
                        nc.scalar.activation(hrelu, hb, AF.Relu)
                        hw = hwork.tile([128, 32, HID], fp32, tag="hw")
                        nc.vector.tensor_mul(hw, hrelu, w2rep)
                        nc.vector.tensor_reduce(
                            apre[h][:, t0:t0 + 32],
                            hw, axis=mybir.AxisListType.X, op=OP.add)

            # ---------------- alpha -> scans -> out ----------------
            for h in range(NH):
                alpha = singles.tile([128, TLOC], fp32, tag=f"alpha{h}")
                nc.scalar.activation(alpha, apre[h], AF.Sigmoid, bias=b2col)
                nc.vector.tensor_scalar(alpha, alpha, 0.01, 0.99,
                                        op0=OP.max, op1=OP.min)
                A_sb = singles.tile([128, TLOC], fp32, tag=f"A{h}")
                nc.vector.tensor_scalar(A_sb, alpha, -1.0, 1.0,
                                        op0=OP.mult, op1=OP.add)
                Bv = singles.tile([128, TLOC], fp32, tag=f"Bv{h}")
                nc.vector.tensor_mul(Bv, alpha, rT[h])
                z_sb = singles.tile([128, TLOC], fp32, tag=f"z{h}")
                nc.vector.tensor_tensor_scan(z_sb, A_sb, Bv, 0.0,
                                             op0=OP.mult, op1=OP.add)
                p_sb = singles.tile([128, TLOC], fp32, tag=f"p{h}")
                nc.vector.tensor_tensor_scan(p_sb, A_sb, ones_sb, 1.0,
                                             op0=OP.mult, op1=OP.mult)
                nc.scalar.dma_start(z_d[h], z_sb)
                nc.scalar.dma_start(p_d[h], p_sb)

    nc.finalize()
    return nc


def _get_program():
    if "nc" not in _CACHE:
        _CACHE["nc"] = _build_program()
    return _CACHE["nc"]


def kernel(r, feat, W1, b1, W2, b2, _run_kwargs=None, _return_results=False):
    from concourse.bass_utils import run_bass_kernel_spmd

    r = np.asarray(r, dtype=np.float32)
    feat = np.asarray(feat, dtype=np.float32)
    W1 = np.asarray(W1, dtype=np.float16)
    b1 = np.asarray(b1, dtype=np.float32).reshape(HID)
    W2 = np.asarray(W2, dtype=np.float32).reshape(HID)
    b2 = np.asarray(b2, dtype=np.float32).reshape(1)

    feat16 = np.ascontiguousarray(
        feat.reshape(T * B, FEAT)).astype(np.float16)
    r2 = r[:, :, 0]

    nc = _get_program()
    in_maps = []
    for c in range(NCORES):
        rt = np.ascontiguousarray(
            r2[c * TLOC:(c + 1) * TLOC, :].T).reshape(NH, 128, TLOC)
        in_maps.append({
            "feat": feat16[c * TLOC * B:(c + 1) * TLOC * B],
            "rt": rt,
            "w1": W1, "b1": b1, "w2": W2, "b2": b2,
        })

    kw = _run_kwargs or {}
    res = run_bass_kernel_spmd(nc, in_maps, core_ids=list(range(NCORES)), **kw)

    # host stitch: y = z + P*carry per slab, carry chain across slabs
    y = np.empty((T, B), dtype=np.float32)
    carry = r2[0].astype(np.float32)
    for c in range(NCORES):
        zc = res.results[c]["z"].transpose(2, 0, 1).reshape(TLOC, B)
        pc = res.results[c]["p"].transpose(2, 0, 1).reshape(TLOC, B)
        y_slab = zc + pc * carry[None, :]
        carry = y_slab[-1]
        y[c * TLOC:(c + 1) * TLOC] = y_slab
    out = y[:, :, None]
    if _return_results:
        return out, res
    return out


# revision 4
# speedup vs baseline: 1.2225x; 1.2225x over previous
"""EMA head kernel for Trainium2 (Bass/Tile), 8 NeuronCores.

Problem: alpha = clip(sigmoid(MLP(feat)), 0.01, 0.99) per (t, b);
         y[0] = r[0]; y[t] = (1-alpha[t])*y[t-1] + alpha[t]*r[t].

Sharding: time dim T=4096 split into 8 slabs of 512 (all B=256 per core).
Each core computes, for its slab, the local affine-scan pieces
    z[t] = A[t]*z[t-1] + Bv[t]   (z[-1] = 0),   A = 1-alpha, Bv = alpha*r
    P[t] = A[t]*P[t-1]           (P[-1] = 1)
and the host stitches slabs with   y = z + P * carry,  carry' = y[-1].
carry_0 = r[0] reproduces y[0] = r[0] exactly: a*r + (1-a)*r = r.

v2 layout: feat is cast to fp16 on the host (halves HBM traffic; the
MLP ran in fp16 already) and kept in natural [t*b, f] order in DRAM.
On-chip it is loaded with HWDGE xbar DMA-transpose straight into
featT [f=128, t*b] SBUF tiles — no PE transposes, no PSUM->SBUF copies.
Each 128-column (t,b)-chunk is one matmul lhsT against W1 (rhs), giving
h [128 bt, 16] in PSUM, collected 32 t-steps per PSUM bank, then
+b1/relu/*W2/reduce on DVE+ACT -> alpha_pre [128 b, t], sigmoid+clip,
and tensor_tensor_scan along the free (t) dim for z and P.
r is transposed host-side to rT [2, 128, t].
"""

import numpy as np

T, B, FEAT, HID = 4096, 256, 128, 16
NCORES = 8
TLOC = T // NCORES  # 512
NH = 2              # batch halves of 128
CH = 16384          # (t,b) columns per feat DMA-transpose chunk (64 t-steps, 4 MB)
NCHUNK = TLOC * B // CH  # 8

_CACHE = {}


def _build_program():
    import concourse.bacc as bacc
    import concourse.bass as bass
    import concourse.tile as tile
    from concourse import mybir

    fp32 = mybir.dt.float32
    fp16 = mybir.dt.float16
    AF = mybir.ActivationFunctionType
    OP = mybir.AluOpType

    nc = bacc.Bacc("TRN2", target_bir_lowering=False, debug=False,
                   num_devices=NCORES)

    feat_d = nc.dram_tensor("feat", [FEAT, TLOC * B], fp16, kind="ExternalInput")
    rt_d = nc.dram_tensor("rt", [NH, 128, TLOC], fp32, kind="ExternalInput")
    w1_d = nc.dram_tensor("w1", [FEAT, HID], fp16, kind="ExternalInput")
    b1_d = nc.dram_tensor("b1", [HID], fp32, kind="ExternalInput")
    w2_d = nc.dram_tensor("w2", [HID], fp32, kind="ExternalInput")
    b2_d = nc.dram_tensor("b2", [1], fp32, kind="ExternalInput")
    z_d = nc.dram_tensor("z", [NH, 128, TLOC], fp32, kind="ExternalOutput")
    p_d = nc.dram_tensor("p", [NH, 128, TLOC], fp32, kind="ExternalOutput")

    with tile.TileContext(nc) as tc:
        with (
            tc.tile_pool(name="singles", bufs=1) as singles,
            tc.tile_pool(name="featin", bufs=3) as featin,
            tc.tile_pool(name="hps", bufs=2, space="PSUM") as hps,
            tc.tile_pool(name="hwork", bufs=2) as hwork,
        ):
            # ---------------- constants / small inputs ----------------
            w1_sb = singles.tile([128, HID], fp16)
            nc.gpsimd.dma_start(w1_sb, w1_d[:, :])
            b1rep = singles.tile([128, 32, HID], fp32)
            nc.gpsimd.dma_start(
                b1rep, bass.AP(b1_d, 0, [[0, 128], [0, 32], [1, HID]]))
            w2rep = singles.tile([128, 32, HID], fp32)
            nc.gpsimd.dma_start(
                w2rep, bass.AP(w2_d, 0, [[0, 128], [0, 32], [1, HID]]))
            b2col = singles.tile([128, 1], fp32)
            nc.gpsimd.dma_start(b2col, bass.AP(b2_d, 0, [[0, 128], [1, 1]]))
            ones_sb = singles.tile([128, TLOC], fp32)
            nc.vector.memset(ones_sb, 1.0)

            rT = [singles.tile([128, TLOC], fp32, tag=f"rT{h}", name=f"rT{h}")
                  for h in range(NH)]
            for h in range(NH):
                nc.scalar.dma_start(rT[h], rt_d[h])

            # per-half alpha_pre accumulators [128 b, t]
            apre = [singles.tile([128, TLOC], fp32, tag=f"apre{h}", name=f"apre{h}")
                    for h in range(NH)]

            # ---------------- main feat pipeline ----------------
            TCH = CH // B  # t-steps per chunk (64)
            for k in range(NCHUNK):
                ft = featin.tile([128, CH], fp16, tag="ft")
                nc.sync.dma_start(ft, feat_d[:, k * CH:(k + 1) * CH])
                # 32-t blocks within this chunk
                for blk in range(TCH // 32):
                    hbank = [hps.tile([128, 32, HID], fp32, tag=f"h{h}",
                                      name=f"hbank{h}")
                             for h in range(NH)]
                    for tt in range(32):
                        col = (blk * 32 + tt) * B
                        for h in range(NH):
                            nc.tensor.matmul(
                                hbank[h][:, tt, :],
                                ft[:, col + h * 128:col + (h + 1) * 128],
                                w1_sb)
                    t0 = k * TCH + blk * 32
                    for h in range(NH):
                        hb = hwork.tile([128, 32, HID], fp32, tag="hb")
                        nc.vector.tensor_add(hb, hbank[h], b1rep)
                        hrelu = hwork.tile([128, 32, HID], fp32, tag="hrelu")
                        nc.scalar.activation(hrelu, hb, AF.Relu)
                        hw = hwork.tile([128, 32, HID], fp32, tag="hw")
                        nc.vector.tensor_mul(hw, hrelu, w2rep)
                        nc.vector.tensor_reduce(
                            apre[h][:, t0:t0 + 32],
                            hw, axis=mybir.AxisListType.X, op=OP.add)

            # ---------------- alpha -> scans -> out ----------------
            for h in range(NH):
                alpha = singles.tile([128, TLOC], fp32, tag=f"alpha{h}")
                nc.scalar.activation(alpha, apre[h], AF.Sigmoid, bias=b2col)
                nc.vector.tensor_scalar(alpha, alpha, 0.01, 0.99,
                                        op0=OP.max, op1=OP.min)
                A_sb = singles.tile([128, TLOC], fp32, tag=f"A{h}")
                nc.vector.tensor_scalar(A_sb, alpha, -1.0, 1.0,
                                        op0=OP.mult, op1=OP.add)
                Bv = singles.tile([128, TLOC], fp32, tag=f"Bv{h}")
                nc.vector.tensor_mul(Bv, alpha, rT[h])
                z_sb = singles.tile([128, TLOC], fp32, tag=f"z{h}")
                nc.vector.tensor_tensor_scan(z_sb, A_sb, Bv, 0.0,
                                             op0=OP.mult, op1=OP.add)
                p_sb = singles.tile([128, TLOC], fp32, tag=f"p{h}")
                nc.vector.tensor_tensor_scan(p_sb, A_sb, ones_sb, 1.0,
                                             op0=OP.mult, op1=OP.mult)
                nc.scalar.dma_start(z_d[h], z_sb)
                nc.scalar.dma_start(p_d[h], p_sb)

    nc.finalize()
    return nc


def _get_program():
    if "nc" not in _CACHE:
        _CACHE["nc"] = _build_program()
    return _CACHE["nc"]


def _host_in_maps(r, feat, W1, b1, W2, b2):
    W1 = np.asarray(W1, dtype=np.float16)
    b1 = np.asarray(b1, dtype=np.float32).reshape(HID)
    W2 = np.asarray(W2, dtype=np.float32).reshape(HID)
    b2 = np.asarray(b2, dtype=np.float32).reshape(1)
    feat16 = np.ascontiguousarray(
        feat.reshape(T * B, FEAT)).astype(np.float16)
    r2 = r[:, :, 0]
    in_maps = []
    BL = 4096  # transpose block: 1 MB input window, L2-resident
    for c in range(NCORES):
        base = c * TLOC * B
        featT = np.empty((FEAT, TLOC * B), np.float16)
        for j in range(0, TLOC * B, BL):
            featT[:, j:j + BL] = feat16[base + j:base + j + BL, :].T
        rt = np.ascontiguousarray(
            r2[c * TLOC:(c + 1) * TLOC, :].T).reshape(NH, 128, TLOC)
        in_maps.append({
            "feat": featT,
            "rt": rt,
            "w1": W1, "b1": b1, "w2": W2, "b2": b2,
        })
    return in_maps


def kernel(r, feat, W1, b1, W2, b2, _run_kwargs=None, _return_results=False):
    from concourse.bass_utils import run_bass_kernel_spmd

    r = np.asarray(r, dtype=np.float32)
    feat = np.asarray(feat, dtype=np.float32)

    nc = _get_program()
    in_maps = _host_in_maps(r, feat, W1, b1, W2, b2)

    kw = _run_kwargs or {}
    res = run_bass_kernel_spmd(nc, in_maps, core_ids=list(range(NCORES)), **kw)

    # host stitch: y = z + P*carry per slab, carry chain across slabs
    y = np.empty((T, B), dtype=np.float32)
    carry = r[0, :, 0].astype(np.float32)
    for c in range(NCORES):
        zc = res.results[c]["z"].transpose(2, 0, 1).reshape(TLOC, B)
        pc = res.results[c]["p"].transpose(2, 0, 1).reshape(TLOC, B)
        y_slab = zc + pc * carry[None, :]
        carry = y_slab[-1]
        y[c * TLOC:(c + 1) * TLOC] = y_slab
    out = y[:, :, None]
    if _return_results:
        return out, res
    return out


# revision 5
# speedup vs baseline: 1.9321x; 1.5804x over previous
"""EMA head kernel for Trainium2 (Bass/Tile), 8 NeuronCores.

Problem: alpha = clip(sigmoid(MLP(feat)), 0.01, 0.99) per (t, b);
         y[0] = r[0]; y[t] = (1-alpha[t])*y[t-1] + alpha[t]*r[t].

Sharding: time dim T=4096 split into 8 slabs of 512 (all B=256 per core).
Each core computes, for its slab, the local affine-scan pieces
    z[t] = A[t]*z[t-1] + Bv[t]   (z[-1] = 0),   A = 1-alpha, Bv = alpha*r
    P[t] = A[t]*P[t-1]           (P[-1] = 1)
and the host stitches slabs with   y = z + P * carry,  carry' = y[-1].
carry_0 = r[0] reproduces y[0] = r[0] exactly: a*r + (1-a)*r = r.

v5: feat is cast to fp16 AND transposed to [f, t*b] on the host, so the
device does plain contiguous HWDGE DMA at full HBM rate and zero on-chip
transposes.  All small constants are host-replicated and loaded as plain
contiguous HWDGE transfers (no SWDGE broadcast descriptors, which stall
the SDMA engines for ~100us).  b1 is folded into the PSUM via a rank-1
PE matmul that initializes each h-bank (start=True), with the per-t-step
matmuls accumulating on top (start=False).  Block epilogue: ACT relu
(PSUM->SBUF), mul by w2 (alternating DVE/GpSimd), DVE reduce ->
alpha_pre [128 b, t]; then sigmoid+clip and tensor_tensor_scan for z/P.
"""

import numpy as np

T, B, FEAT, HID = 4096, 256, 128, 16
NCORES = 8
TLOC = T // NCORES  # 512
NH = 2              # batch halves of 128
CH = 16384          # (t,b) columns per feat chunk (64 t-steps, 4 MB fp16)
NCHUNK = TLOC * B // CH  # 8

_CACHE = {}


def _build_program():
    import concourse.bacc as bacc
    import concourse.bass as bass
    import concourse.tile as tile
    from concourse import mybir

    fp32 = mybir.dt.float32
    fp16 = mybir.dt.float16
    AF = mybir.ActivationFunctionType
    OP = mybir.AluOpType

    nc = bacc.Bacc("TRN2", target_bir_lowering=False, debug=False,
                   num_devices=NCORES)

    feat_d = nc.dram_tensor("feat", [FEAT, TLOC * B], fp16, kind="ExternalInput")
    rt_d = nc.dram_tensor("rt", [NH, 128, TLOC], fp32, kind="ExternalInput")
    w1_d = nc.dram_tensor("w1", [FEAT, HID], fp16, kind="ExternalInput")
    b1r_d = nc.dram_tensor("b1r", [1, 32 * HID], fp16, kind="ExternalInput")
    w2rep_d = nc.dram_tensor("w2rep", [128, 32 * HID], fp32, kind="ExternalInput")
    b2col_d = nc.dram_tensor("b2col", [128, 1], fp32, kind="ExternalInput")
    z_d = nc.dram_tensor("z", [NH, 128, TLOC], fp32, kind="ExternalOutput")
    p_d = nc.dram_tensor("p", [NH, 128, TLOC], fp32, kind="ExternalOutput")

    with tile.TileContext(nc) as tc:
        with (
            tc.tile_pool(name="singles", bufs=1) as singles,
            tc.tile_pool(name="featin", bufs=3) as featin,
            tc.tile_pool(name="hps", bufs=2, space="PSUM") as hps,
            tc.tile_pool(name="hwork", bufs=2) as hwork,
        ):
            # ------------- constants / small inputs (all HWDGE) -------------
            w1_sb = singles.tile([128, HID], fp16)
            nc.scalar.dma_start(w1_sb, w1_d[:, :])
            b1row = singles.tile([1, 32 * HID], fp16)
            nc.scalar.dma_start(b1row, b1r_d[:, :])
            ones1 = singles.tile([1, 128], fp16)
            nc.vector.memset(ones1, 1.0)
            w2rep = singles.tile([128, 32, HID], fp32)
            nc.scalar.dma_start(
                w2rep, w2rep_d[:, :].rearrange("p (t h) -> p t h", h=HID))
            b2col = singles.tile([128, 1], fp32)
            nc.scalar.dma_start(b2col, b2col_d[:, :])
            ones_sb = singles.tile([128, TLOC], fp32)
            nc.vector.memset(ones_sb, 1.0)

            rT = [singles.tile([128, TLOC], fp32, tag=f"rT{h}", name=f"rT{h}")
                  for h in range(NH)]
            for h in range(NH):
                nc.scalar.dma_start(rT[h], rt_d[h])

            # per-half alpha_pre accumulators [128 b, t]
            apre = [singles.tile([128, TLOC], fp32, tag=f"apre{h}", name=f"apre{h}")
                    for h in range(NH)]

            # ---------------- main feat pipeline ----------------
            TCH = CH // B  # t-steps per chunk (64)
            mul_parity = 0
            for k in range(NCHUNK):
                ft = featin.tile([128, CH], fp16, tag="ft")
                if k == 0:
                    # split the first chunk to start compute sooner
                    for q in range(4):
                        nc.sync.dma_start(
                            ft[:, q * (CH // 4):(q + 1) * (CH // 4)],
                            feat_d[:, q * (CH // 4):(q + 1) * (CH // 4)])
                else:
                    nc.sync.dma_start(ft, feat_d[:, k * CH:(k + 1) * CH])
                # 32-t blocks within this chunk
                for blk in range(TCH // 32):
                    hbank = [hps.tile([128, 32, HID], fp32, tag=f"h{h}",
                                      name=f"hbank{h}")
                             for h in range(NH)]
                    for h in range(NH):
                        # rank-1 bias: psum[:, t, hid] = b1[hid]
                        nc.tensor.matmul(hbank[h][:, :, :], ones1, b1row,
                                         start=True, stop=False,
                                         skip_group_check=True)
                    for tt in range(32):
                        col = (blk * 32 + tt) * B
                        for h in range(NH):
                            nc.tensor.matmul(
                                hbank[h][:, tt, :],
                                ft[:, col + h * 128:col + (h + 1) * 128],
                                w1_sb, start=False, stop=True,
                                skip_group_check=True)
                    t0 = k * TCH + blk * 32
                    for h in range(NH):
                        hrelu = hwork.tile([128, 32, HID], fp32, tag="hrelu")
                        nc.scalar.activation(hrelu, hbank[h], AF.Relu)
                        hw = hwork.tile([128, 32, HID], fp32, tag="hw")
                        if mul_parity == 0:
                            nc.vector.tensor_mul(hw, hrelu, w2rep)
                        else:
                            nc.gpsimd.tensor_mul(hw, hrelu, w2rep)
                        mul_parity ^= 1
                        nc.vector.tensor_reduce(
                            apre[h][:, t0:t0 + 32],
                            hw, axis=mybir.AxisListType.X, op=OP.add)

            # ---------------- alpha -> scans -> out ----------------
            for h in range(NH):
                alpha = singles.tile([128, TLOC], fp32, tag=f"alpha{h}")
                nc.scalar.activation(alpha, apre[h], AF.Sigmoid, bias=b2col)
                nc.vector.tensor_scalar(alpha, alpha, 0.01, 0.99,
                                        op0=OP.max, op1=OP.min)
                A_sb = singles.tile([128, TLOC], fp32, tag=f"A{h}")
                nc.vector.tensor_scalar(A_sb, alpha, -1.0, 1.0,
                                        op0=OP.mult, op1=OP.add)
                Bv = singles.tile([128, TLOC], fp32, tag=f"Bv{h}")
                nc.gpsimd.tensor_mul(Bv, alpha, rT[h])
                z_sb = singles.tile([128, TLOC], fp32, tag=f"z{h}")
                nc.vector.tensor_tensor_scan(z_sb, A_sb, Bv, 0.0,
                                             op0=OP.mult, op1=OP.add)
                p_sb = singles.tile([128, TLOC], fp32, tag=f"p{h}")
                nc.vector.tensor_tensor_scan(p_sb, A_sb, ones_sb, 1.0,
                                             op0=OP.mult, op1=OP.mult)
                nc.scalar.dma_start(z_d[h], z_sb)
                nc.scalar.dma_start(p_d[h], p_sb)

    nc.finalize()
    return nc


def _get_program():
    if "nc" not in _CACHE:
        _CACHE["nc"] = _build_program()
    return _CACHE["nc"]


def _host_in_maps(r, feat, W1, b1, W2, b2):
    W1 = np.asarray(W1, dtype=np.float16)
    b1 = np.asarray(b1, dtype=np.float32).reshape(HID)
    W2 = np.asarray(W2, dtype=np.float32).reshape(HID)
    b2 = np.asarray(b2, dtype=np.float32).reshape(1)
    b1r = np.ascontiguousarray(
        np.tile(b1.astype(np.float16), 32)[None, :])
    w2rep = np.ascontiguousarray(
        np.broadcast_to(np.tile(W2, 32)[None, :], (128, 32 * HID)))
    b2col = np.ascontiguousarray(np.broadcast_to(b2[None, :], (128, 1)))
    feat16 = np.ascontiguousarray(
        feat.reshape(T * B, FEAT)).astype(np.float16)
    r2 = r[:, :, 0]
    in_maps = []
    BL = 4096  # transpose block: 1 MB input window, L2-resident
    for c in range(NCORES):
        base = c * TLOC * B
        featT = np.empty((FEAT, TLOC * B), np.float16)
        for j in range(0, TLOC * B, BL):
            featT[:, j:j + BL] = feat16[base + j:base + j + BL, :].T
        rt = np.ascontiguousarray(
            r2[c * TLOC:(c + 1) * TLOC, :].T).reshape(NH, 128, TLOC)
        in_maps.append({
            "feat": featT,
            "rt": rt,
            "w1": W1, "b1r": b1r, "w2rep": w2rep, "b2col": b2col,
        })
    return in_maps


def kernel(r, feat, W1, b1, W2, b2, _run_kwargs=None, _return_results=False):
    from concourse.bass_utils import run_bass_kernel_spmd

    r = np.asarray(r, dtype=np.float32)
    feat = np.asarray(feat, dtype=np.float32)

    nc = _get_program()
    in_maps = _host_in_maps(r, feat, W1, b1, W2, b2)

    kw = _run_kwargs or {}
    res = run_bass_kernel_spmd(nc, in_maps, core_ids=list(range(NCORES)), **kw)

    # host stitch: y = z + P*carry per slab, carry chain across slabs
    y = np.empty((T, B), dtype=np.float32)
    carry = r[0, :, 0].astype(np.float32)
    for c in range(NCORES):
        zc = res.results[c]["z"].transpose(2, 0, 1).reshape(TLOC, B)
        pc = res.results[c]["p"].transpose(2, 0, 1).reshape(TLOC, B)
        y_slab = zc + pc * carry[None, :]
        carry = y_slab[-1]
        y[c * TLOC:(c + 1) * TLOC] = y_slab
    out = y[:, :, None]
    if _return_results:
        return out, res
    return out
